# revision 13
# baseline (speedup 1.0000x reference)
"""BitNetLinear (ternary eval-mode) forward on 8 trn2 NeuronCores.

Math (reference):
    s_w  = max(mean|W|, eps);  q = sign(W) * (|W/s_w| > 0.5)
    s_x  = max(mean|x|, eps)
    out  = (x/s_x) @ (q*s_w)^T * s_x + bias * s_x
         = x @ q^T * s_w + bias * s_x          (exact in real arithmetic)

Sharding: 2D grid, TG=4 token groups x FG=2 out-feature groups.
Each core: T=1024 tokens, O=2048 out features, I=4096 contraction.
Host passes x and W shards pre-transposed (i-major) with x cast to
fp8-e4m3 (measured end-to-end max rel err 0.017 < 2e-2 gate); W stays
f32 so the |w|>thr comparisons are exact.

Device pipeline per core (fp8 DoubleRow matmul):
  - phase S: |W| partial sum over this core's i-slab, read as 8 x 1MiB
    DMAs split across both hwdge rings with nothing else in flight (the
    slab doubles as the kb<4 strips of every o-chunk, so the 8 MB read
    is reused by quantization). Rolling DVE abs-reduce, gpsimd
    partition_all_reduce, then a 1-scalar ncfw AllReduce — the only
    collective; its ~50-90 us init/barrier/latency dominates the head,
    so all x / chunk-0 W prefetch is gated behind the reduction and
    runs on the ACT ring underneath the collective. The thr broadcast
    is issued after every prefetch so it cannot head-of-line-block a
    dma queue while it waits on the collective.
  - thr = 0.5*max(sum/N, eps) on chip.
  - quantize per [128,2,512] f32 strip into q in {-1,0,1} fp8 (2 ops;
    fp8 stores are only fast from f32 sources, and only on DVE/ACT):
        l  = Sigmoid(-1e30*(w + thr))     (ACT; exact {0,1} step)
        q8 = (w is_gt thr) - l            (DVE scalar_tensor_tensor)
  - matmul: fp8e4 DoubleRow, stationary q8 [128,2,128], moving x
    [128,2,512] -> psum [128 o, 512 t], 256-deep contraction per MM
    (measured 218.8 ns/MM = 2x bf16 FLOP rate, LDWEIGHTS fully hidden).
  - evict with scale s_w = 2*thr on ACT; output is o-major [O, T];
    host transposes back (layout only).
"""

import sys

sys.path.insert(0, "/opt/trn_rl_repo")

import numpy as np

P = 128
EPS = 1e-8
BIG = 1.0e30  # sigmoid step sharpener

B, S = 2, 2048
I_FULL = 4096  # in_features
O_FULL = 4096  # out_features
N_CORES = 8
TG, FG = 4, 2
T_SH = (B * S) // TG  # 1024
O_SH = O_FULL // FG  # 2048
KB = 256  # contraction rows per DoubleRow k-block
OC = 512  # o-chunk width (quantize granularity)


def build_nc(T, O, I, n_cores, tg, w_elems_total):
    """Build + compile the SPMD Bass module for one core shape."""
    from concourse import bacc, mybir, tile, bass_isa
    import concourse.bass as bass
    from concourse.bass import ts, ds

    f32 = mybir.dt.float32
    fp8 = mybir.dt.float8e4
    A = mybir.AluOpType
    DR = mybir.MatmulPerfMode.DoubleRow

    assert T % OC == 0 and O % OC == 0 and I % KB == 0
    n_kb = I // KB  # 16 k-blocks
    n_oc = O // OC  # 4 o-chunks
    n_th = T // OC  # 2 token halves
    n_j = OC // P  # 4 o-blocks per chunk
    slab_kb = (I // tg) // KB  # 4 k-blocks in this core's |W| slab

    nc = bacc.Bacc(
        "TRN2", target_bir_lowering=False, debug=False, num_devices=n_cores
    )
    xT = nc.dram_tensor("xT", [I, T], fp8, kind="ExternalInput").ap()
    wT = nc.dram_tensor("wT", [I, O], f32, kind="ExternalInput").ap()
    out_sh = nc.dram_tensor("out_sh", [O, T], f32, kind="ExternalOutput").ap()

    def src3(t_ap, off, d0, d1, d2):
        # 3D DRAM access pattern [(stride, count), ...], element units
        return bass.AP(
            tensor=t_ap.tensor,
            offset=t_ap.offset + off,
            ap=[list(d0), list(d1), list(d2)],
        )

    def w_src(kb, c):
        return src3(wT, kb * KB * O + c * OC, (O, P), (P * O, 2), (1, OC))

    with tile.TileContext(nc) as tc:
        with (
            tc.tile_pool(name="scal", bufs=1) as scal,
            tc.tile_pool(name="dram", bufs=1, space="DRAM") as dram,
            tc.tile_pool(name="slab", bufs=1) as slab_pool,
            tc.tile_pool(name="xp", bufs=1) as x_pool,
            tc.tile_pool(name="wsp", bufs=1) as ws_pool,
            tc.tile_pool(name="lp", bufs=1) as l_pool,
            tc.tile_pool(name="qp", bufs=1) as q_pool,
            tc.tile_pool(name="op", bufs=3) as o_pool,
            tc.tile_pool(name="ps", bufs=1, space="PSUM") as ps_pool,
        ):
            # ---- phase S: |W| partial over the slab (k-blocks 0..slab_kb
            # of every o-chunk; the host i-rolls wT per core so the slab
            # rows are distinct across the token group). Strips stay
            # resident — they are exactly the kb<slab_kb strips the
            # quantizer needs later.
            slab = {}
            OH = O // 2
            acc = scal.tile([P, 2 * slab_kb], f32)
            for j in range(2 * slab_kb):
                kb, oh = j // 2, j % 2
                wst = slab_pool.tile(
                    [P, 2, OH], f32, tag=f"sl_{j}", name=f"sl_{j}"
                )
                # >=1MiB DMAs (each splits across its ring's 16 SDMA
                # slots), alternating the two hwdge rings; nothing else
                # is allowed on either ring until the slab lands
                eng = nc.sync if j % 2 == 0 else nc.scalar
                eng.dma_start(
                    wst[:],
                    src3(wT, kb * KB * O + oh * OH, (O, P), (P * O, 2), (1, OH)),
                )
                nc.vector.tensor_reduce(
                    acc[:, j : j + 1],
                    wst[:],
                    axis=mybir.AxisListType.XY,
                    op=A.add,
                    apply_absolute_value=True,
                )
                slab[j] = wst
            red = scal.tile([P, 1], f32)
            nc.vector.tensor_reduce(red[:], acc[:], axis=mybir.AxisListType.X, op=A.add)
            ssum = scal.tile([P, 1], f32)
            nc.gpsimd.partition_all_reduce(
                ssum[:], red[:], channels=P, reduce_op=bass_isa.ReduceOp.add
            )

            # ---- the one collective: AllReduce of the scalar partial.
            # cc_in rides the gpsimd queue: the earlier the trigger, the
            # earlier this core enters the ncfw barrier (its end time
            # tracks the trigger), and the sync queue stays slab-only.
            cc_in = dram.tile([1, 1], f32)
            cc_out = dram.tile([1, 1], f32)
            nc.sync.dma_start(cc_in[:], ssum[0:1, 0:1])
            nc.gpsimd.collective_compute(
                "AllReduce",
                A.add,
                replica_groups=[list(range(n_cores))],
                ins=[cc_in[:]],
                outs=[cc_out[:]],
            )
            co = cc_out[:]

            # ---- x prefetch on the ACT ring, gated behind the slab
            # reduction: transfers within a ring overlap (FIFO is
            # start-order only), so an explicit dependency is the only
            # way to keep the slab read at full dual-ring bandwidth.
            gate = scal.tile([P, 1], f32)
            nc.scalar.activation(gate[:], red[:], mybir.ActivationFunctionType.Copy)
            x8 = []
            for kb in range(n_kb):
                xb = x_pool.tile([P, 2, T], fp8, tag=f"x_{kb}", name=f"x_{kb}")
                nc.scalar.dma_start(
                    xb[:], src3(xT, kb * KB * T, (T, P), (P * T, 2), (1, T))
                )
                x8.append(xb)

            # ---- chunk-0 streamed-strip prefetch (runs under the
            # collective head; WAR rotation handles chunks 1-3)
            def stream_strip(ci, kb, engine=None):
                wst = ws_pool.tile(
                    [P, 2, OC], f32, tag=f"ws_{kb}", name=f"ws_{ci}_{kb}"
                )
                (engine or nc.sync).dma_start(wst[:], w_src(kb, ci))
                return wst

            strips0 = {
                kb: stream_strip(0, kb, nc.scalar) for kb in range(slab_kb, n_kb)
            }

            # ---- thr broadcast, AFTER every prefetch issue in program
            # order (it waits on the collective; anything behind it on a
            # dma queue would head-of-line block until the AR completes)
            s_bc = scal.tile([P, 1], f32)
            nc.sync.dma_start(
                s_bc[:],
                bass.AP(tensor=co.tensor, offset=co.offset, ap=[[0, P], [1, 1]]),
            )
            # thr = 0.5*max(sum/N, EPS); bstep = -BIG*thr; swt = s_w = 2*thr.
            # All three directly from s_bc in one op each (parallel, off the
            # critical chain). N = 2^24 so the x2/x0.5 foldings are exact.
            thr = scal.tile([P, 1], f32)
            nc.vector.tensor_scalar(
                out=thr[:],
                in0=s_bc[:],
                scalar1=0.5 / float(w_elems_total),
                scalar2=0.5 * EPS,
                op0=A.mult,
                op1=A.max,
            )
            bstep = scal.tile([P, 1], f32)
            nc.vector.tensor_scalar(
                out=bstep[:],
                in0=s_bc[:],
                scalar1=-BIG * 0.5 / float(w_elems_total),
                scalar2=-BIG * 0.5 * EPS,
                op0=A.mult,
                op1=A.min,
            )
            swt = scal.tile([P, 1], f32)
            nc.vector.tensor_scalar(
                out=swt[:],
                in0=s_bc[:],
                scalar1=1.0 / float(w_elems_total),
                scalar2=EPS,
                op0=A.mult,
                op1=A.max,
            )

            # ---- main loop
            for ci in range(n_oc):
                qts = []
                for kb in range(n_kb):
                    if kb < slab_kb:
                        j = kb * 2 + ci // 2
                        wst = slab[j][:, :, ds((ci % 2) * OC, OC)]
                    elif ci == 0:
                        wst = strips0[kb][:]
                    else:
                        wst = stream_strip(ci, kb)[:]
                    # l = 1 iff w < -thr (sigmoid of -BIG*(w+thr): exact step)
                    lt = l_pool.tile(
                        [P, 2, OC], f32, tag=f"l_{kb % 3}", name=f"l_{ci}_{kb}"
                    )
                    nc.scalar.activation(
                        lt[:],
                        wst,
                        mybir.ActivationFunctionType.Sigmoid,
                        scale=-BIG,
                        bias=bstep[:],
                    )
                    # q8 = (w > thr) - l  in {-1, 0, 1}, straight to fp8
                    q8 = q_pool.tile(
                        [P, 2, OC], fp8, tag=f"q_{kb}_{ci % 2}", name=f"q_{ci}_{kb}"
                    )
                    nc.vector.scalar_tensor_tensor(
                        out=q8[:],
                        in0=wst,
                        scalar=thr[:],
                        op0=A.is_gt,
                        in1=lt[:],
                        op1=A.subtract,
                    )
                    qts.append(q8)
                for j in range(n_oc):
                    ob = ci * 4 + j
                    pst = [
                        ps_pool.tile(
                            [P, OC],
                            f32,
                            tag=f"ps{(ob * n_th + h) % 8}",
                            name=f"ps_{ob}_{h}",
                        )
                        for h in range(n_th)
                    ]
                    for kb in range(n_kb):
                        lhs = qts[kb][:, :, ts(j, P)]
                        for h in range(n_th):
                            nc.tensor.matmul(
                                pst[h][:],
                                lhsT=lhs,
                                rhs=x8[kb][:, :, ds(h * OC, OC)],
                                start=(kb == 0),
                                stop=(kb == n_kb - 1),
                                perf_mode=DR,
                            )
                    for h in range(n_th):
                        osb = o_pool.tile([P, OC], f32, tag="o", name=f"o_{ob}_{h}")
                        # psum holds x @ q^T; scale by s_w
                        nc.scalar.activation(
                            osb[:],
                            pst[h][:],
                            mybir.ActivationFunctionType.Copy,
                            scale=swt[:],
                        )
                        nc.scalar.dma_start(out_sh[ts(ob, P), ds(h * OC, OC)], osb[:])

    nc.compile()
    return nc


_CACHE = {}


def _get_nc(key):
    if key not in _CACHE:
        _CACHE[key] = build_nc(*key)
    return _CACHE[key]


def make_in_maps(x2d, weight, n_cores=N_CORES, tg=TG, fg=FG):
    """Host-side sharding: per-core pre-transposed inputs, x in fp8-e4m3."""
    import ml_dtypes

    t_tot, i_full = x2d.shape
    o_full = weight.shape[0]
    t_sh = t_tot // tg
    o_sh = o_full // fg
    i_slab = i_full // tg
    x_f8 = x2d.astype(ml_dtypes.float8_e4m3)
    wT_halves = {}
    for b in range(fg):
        wT_halves[b] = np.ascontiguousarray(weight[b * o_sh : (b + 1) * o_sh].T)
    in_maps = []
    for cid in range(n_cores):
        g, b = cid // fg, cid % fg
        # rotate i-rows so rows [0, i_slab) are this core's distinct |W|
        # slab; the contraction sum is invariant to the rotation as long
        # as xT rows are rotated identically.
        roll = -g * i_slab
        in_maps.append(
            {
                "xT": np.ascontiguousarray(
                    np.roll(x_f8[g * t_sh : (g + 1) * t_sh].T, roll, axis=0)
                ),
                "wT": np.roll(wT_halves[b], roll, axis=0),
            }
        )
    return in_maps


def run(x2d, weight, n_cores=N_CORES, tg=TG, fg=FG):
    """Run the sharded device computation: returns x @ q^T * s_w, [Ttot, O_full]."""
    from concourse.bass_utils import run_bass_kernel_spmd

    t_tot, i_full = x2d.shape
    o_full = weight.shape[0]
    t_sh = t_tot // tg
    o_sh = o_full // fg
    key = (t_sh, o_sh, i_full, n_cores, tg, o_full * i_full)
    nc = _get_nc(key)

    in_maps = make_in_maps(x2d, weight, n_cores, tg, fg)
    res = run_bass_kernel_spmd(nc, in_maps, core_ids=list(range(n_cores)))
    out = np.empty((t_tot, o_full), np.float32)
    for cid in range(n_cores):
        g, b = cid // fg, cid % fg
        out[g * t_sh : (g + 1) * t_sh, b * o_sh : (b + 1) * o_sh] = res.results[
            cid
        ]["out_sh"].T
    return out


def kernel(x, weight, bias):
    x = np.asarray(x, np.float32)
    weight = np.asarray(weight, np.float32)
    bias = np.asarray(bias, np.float32)
    t_tot = x.shape[0] * x.shape[1]
    out = run(x.reshape(t_tot, x.shape[2]), weight)
    # bias term: out += bias * s_x (exact reference semantics; zero for
    # this problem's bias). The matmul term is s_x-invariant.
    if np.any(bias):
        s_x = np.float32(max(np.mean(np.abs(x)), EPS))
        out = out + (bias * s_x)[None, :]
    return out.reshape(x.shape[0], x.shape[1], weight.shape[0])


# revision 14
# speedup vs baseline: 1.1629x; 1.1629x over previous
"""BitNetLinear (ternary eval-mode) forward on 8 trn2 NeuronCores.

Math (reference):
    s_w  = max(mean|W|, eps);  q = sign(W) * (|W/s_w| > 0.5)
    s_x  = max(mean|x|, eps)
    out  = (x/s_x) @ (q*s_w)^T * s_x + bias * s_x
         = x @ q^T * s_w + bias * s_x          (exact in real arithmetic)

Sharding: 2D grid, TG=4 token groups x FG=2 out-feature groups.
Each core: T=1024 tokens, O=2048 out features, I=4096 contraction.
Host passes x and W shards pre-transposed (i-major) with x cast to
fp8-e4m3 (measured end-to-end max rel err 0.017 < 2e-2 gate); W stays
f32 so the |w|>thr comparisons are exact.

Device pipeline per core (fp8 DoubleRow matmul):
  - phase S: |W| partial sum over this core's i-slab, read as 8 x 1MiB
    DMAs split across both hwdge rings with nothing else in flight (the
    slab doubles as the kb<4 strips of every o-chunk, so the 8 MB read
    is reused by quantization). Rolling DVE abs-reduce, gpsimd
    partition_all_reduce, then a 1-scalar ncfw AllReduce — the only
    collective; its ~50-90 us init/barrier/latency dominates the head,
    so all x / chunk-0 W prefetch is gated behind the reduction and
    runs on the ACT ring underneath the collective. The thr broadcast
    is issued after every prefetch so it cannot head-of-line-block a
    dma queue while it waits on the collective.
  - thr = 0.5*max(sum/N, eps) on chip.
  - quantize per [128,2,512] f32 strip into q in {-1,0,1} fp8 (2 ops;
    fp8 stores are only fast from f32 sources, and only on DVE/ACT):
        l  = Sigmoid(-1e30*(w + thr))     (ACT; exact {0,1} step)
        q8 = (w is_gt thr) - l            (DVE scalar_tensor_tensor)
  - matmul: fp8e4 DoubleRow, stationary q8 [128,2,128], moving x
    [128,2,512] -> psum [128 o, 512 t], 256-deep contraction per MM
    (measured 218.8 ns/MM = 2x bf16 FLOP rate, LDWEIGHTS fully hidden).
  - evict with scale s_w = 2*thr on ACT; output is o-major [O, T];
    host transposes back (layout only).
"""

import sys

sys.path.insert(0, "/opt/trn_rl_repo")

import numpy as np

P = 128
EPS = 1e-8
BIG = 1.0e30  # sigmoid step sharpener

B, S = 2, 2048
I_FULL = 4096  # in_features
O_FULL = 4096  # out_features
N_CORES = 8
TG, FG = 4, 2
T_SH = (B * S) // TG  # 1024
O_SH = O_FULL // FG  # 2048
KB = 256  # contraction rows per DoubleRow k-block
OC = 512  # o-chunk width (quantize granularity)


def build_nc(T, O, I, n_cores, tg, w_elems_total):
    """Build + compile the SPMD Bass module for one core shape."""
    from concourse import bacc, mybir, tile, bass_isa
    import concourse.bass as bass
    from concourse.bass import ts, ds

    f32 = mybir.dt.float32
    fp8 = mybir.dt.float8e4
    A = mybir.AluOpType
    DR = mybir.MatmulPerfMode.DoubleRow

    assert T % OC == 0 and O % OC == 0 and I % KB == 0
    n_kb = I // KB  # 16 k-blocks
    n_oc = O // OC  # 4 o-chunks
    n_th = T // OC  # 2 token halves
    slab_kb = (I // tg) // KB  # 4 k-blocks in this core's |W| slab

    nc = bacc.Bacc(
        "TRN2", target_bir_lowering=False, debug=False, num_devices=n_cores
    )
    xT = nc.dram_tensor("xT", [I, T], fp8, kind="ExternalInput").ap()
    wT = nc.dram_tensor("wT", [I, O], f32, kind="ExternalInput").ap()
    out_sh = nc.dram_tensor("out_sh", [O, T], f32, kind="ExternalOutput").ap()

    def src3(t_ap, off, d0, d1, d2):
        # 3D DRAM access pattern [(stride, count), ...], element units
        return bass.AP(
            tensor=t_ap.tensor,
            offset=t_ap.offset + off,
            ap=[list(d0), list(d1), list(d2)],
        )

    def w_src(kb, c):
        return src3(wT, kb * KB * O + c * OC, (O, P), (P * O, 2), (1, OC))

    with tile.TileContext(nc) as tc:
        with (
            tc.tile_pool(name="scal", bufs=1) as scal,
            tc.tile_pool(name="dram", bufs=1, space="DRAM") as dram,
            tc.tile_pool(name="slab", bufs=1) as slab_pool,
            tc.tile_pool(name="xp", bufs=1) as x_pool,
            tc.tile_pool(name="wsp", bufs=1) as ws_pool,
            tc.tile_pool(name="lp", bufs=1) as l_pool,
            tc.tile_pool(name="qp", bufs=1) as q_pool,
            tc.tile_pool(name="op", bufs=3) as o_pool,
            tc.tile_pool(name="ps", bufs=1, space="PSUM") as ps_pool,
        ):
            # ---- phase S: |W| partial over the slab (k-blocks 0..slab_kb
            # of every o-chunk; the host i-rolls wT per core so the slab
            # rows are distinct across the token group). Strips stay
            # resident — they are exactly the kb<slab_kb strips the
            # quantizer needs later.
            slab = {}
            OH = O // 2
            acc = scal.tile([P, 2 * slab_kb], f32)
            for j in range(2 * slab_kb):
                kb, oh = j // 2, j % 2
                wst = slab_pool.tile(
                    [P, 2, OH], f32, tag=f"sl_{j}", name=f"sl_{j}"
                )
                # >=1MiB DMAs (each splits across its ring's 16 SDMA
                # slots), alternating the two hwdge rings; nothing else
                # is allowed on either ring until the slab lands
                eng = nc.sync if j % 2 == 0 else nc.scalar
                eng.dma_start(
                    wst[:],
                    src3(wT, kb * KB * O + oh * OH, (O, P), (P * O, 2), (1, OH)),
                )
                nc.vector.tensor_reduce(
                    acc[:, j : j + 1],
                    wst[:],
                    axis=mybir.AxisListType.XY,
                    op=A.add,
                    apply_absolute_value=True,
                )
                slab[j] = wst
            red = scal.tile([P, 1], f32)
            nc.vector.tensor_reduce(red[:], acc[:], axis=mybir.AxisListType.X, op=A.add)
            ssum = scal.tile([P, 1], f32)
            nc.gpsimd.partition_all_reduce(
                ssum[:], red[:], channels=P, reduce_op=bass_isa.ReduceOp.add
            )

            # ---- the one collective: AllReduce of the scalar partial
            # (triggered as early as possible; the ncfw barrier's per-core
            # end time is runtime jitter we cannot control)
            cc_in = dram.tile([1, 1], f32)
            cc_out = dram.tile([1, 1], f32)
            nc.sync.dma_start(cc_in[:], ssum[0:1, 0:1])
            nc.gpsimd.collective_compute(
                "AllReduce",
                A.add,
                replica_groups=[list(range(n_cores))],
                ins=[cc_in[:]],
                outs=[cc_out[:]],
            )
            co = cc_out[:]

            # ---- x prefetch on the ACT ring, gated behind the slab
            # reduction: transfers within a ring overlap (FIFO is
            # start-order only), so an explicit dependency is the only
            # way to keep the slab read at full dual-ring bandwidth.
            gate = scal.tile([P, 1], f32)
            nc.scalar.activation(gate[:], red[:], mybir.ActivationFunctionType.Copy)
            x8 = []
            for kb in range(n_kb):
                xb = x_pool.tile([P, 2, T], fp8, tag=f"x_{kb}", name=f"x_{kb}")
                nc.scalar.dma_start(
                    xb[:], src3(xT, kb * KB * T, (T, P), (P * T, 2), (1, T))
                )
                x8.append(xb)

            # ---- chunk-0 streamed-strip prefetch (runs under the
            # collective head; WAR rotation handles chunks 1-3)
            def stream_strip(ci, kb, engine=None):
                wst = ws_pool.tile(
                    [P, 2, OC], f32, tag=f"ws_{kb}", name=f"ws_{ci}_{kb}"
                )
                (engine or nc.sync).dma_start(wst[:], w_src(kb, ci))
                return wst

            strips0 = {
                kb: stream_strip(0, kb, nc.scalar) for kb in range(slab_kb, n_kb)
            }

            # ---- thr broadcast, AFTER every prefetch issue in program
            # order (it waits on the collective; anything behind it on a
            # dma queue would head-of-line block until the AR completes)
            s_bc = scal.tile([P, 1], f32)
            nc.sync.dma_start(
                s_bc[:],
                bass.AP(tensor=co.tensor, offset=co.offset, ap=[[0, P], [1, 1]]),
            )
            # thr = 0.5*max(sum/N, EPS); bstep = -BIG*thr; swt = s_w = 2*thr.
            # All three directly from s_bc in one op each (parallel, off the
            # critical chain). N = 2^24 so the x2/x0.5 foldings are exact.
            thr = scal.tile([P, 1], f32)
            nc.vector.tensor_scalar(
                out=thr[:],
                in0=s_bc[:],
                scalar1=0.5 / float(w_elems_total),
                scalar2=0.5 * EPS,
                op0=A.mult,
                op1=A.max,
            )
            bstep = scal.tile([P, 1], f32)
            nc.vector.tensor_scalar(
                out=bstep[:],
                in0=s_bc[:],
                scalar1=-BIG * 0.5 / float(w_elems_total),
                scalar2=-BIG * 0.5 * EPS,
                op0=A.mult,
                op1=A.min,
            )
            swt = scal.tile([P, 1], f32)
            nc.vector.tensor_scalar(
                out=swt[:],
                in0=s_bc[:],
                scalar1=1.0 / float(w_elems_total),
                scalar2=EPS,
                op0=A.mult,
                op1=A.max,
            )

            # ---- main loop
            for ci in range(n_oc):
                qts = []
                for kb in range(n_kb):
                    if kb < slab_kb:
                        j = kb * 2 + ci // 2
                        wst = slab[j][:, :, ds((ci % 2) * OC, OC)]
                    elif ci == 0:
                        wst = strips0[kb][:]
                    else:
                        wst = stream_strip(ci, kb)[:]
                    # l = 1 iff w < -thr (sigmoid of -BIG*(w+thr): exact step)
                    lt = l_pool.tile(
                        [P, 2, OC], f32, tag=f"l_{kb % 3}", name=f"l_{ci}_{kb}"
                    )
                    nc.scalar.activation(
                        lt[:],
                        wst,
                        mybir.ActivationFunctionType.Sigmoid,
                        scale=-BIG,
                        bias=bstep[:],
                    )
                    # q8 = (w > thr) - l  in {-1, 0, 1}, straight to fp8
                    q8 = q_pool.tile(
                        [P, 2, OC], fp8, tag=f"q_{kb}_{ci % 2}", name=f"q_{ci}_{kb}"
                    )
                    nc.vector.scalar_tensor_tensor(
                        out=q8[:],
                        in0=wst,
                        scalar=thr[:],
                        op0=A.is_gt,
                        in1=lt[:],
                        op1=A.subtract,
                    )
                    qts.append(q8)
                for j in range(n_oc):
                    ob = ci * 4 + j
                    pst = [
                        ps_pool.tile(
                            [P, OC],
                            f32,
                            tag=f"ps{(ob * n_th + h) % 8}",
                            name=f"ps_{ob}_{h}",
                        )
                        for h in range(n_th)
                    ]
                    for kb in range(n_kb):
                        lhs = qts[kb][:, :, ts(j, P)]
                        for h in range(n_th):
                            nc.tensor.matmul(
                                pst[h][:],
                                lhsT=lhs,
                                rhs=x8[kb][:, :, ds(h * OC, OC)],
                                start=(kb == 0),
                                stop=(kb == n_kb - 1),
                                perf_mode=DR,
                            )
                    for h in range(n_th):
                        osb = o_pool.tile([P, OC], f32, tag="o", name=f"o_{ob}_{h}")
                        # psum holds x @ q^T; scale by s_w
                        nc.scalar.activation(
                            osb[:],
                            pst[h][:],
                            mybir.ActivationFunctionType.Copy,
                            scale=swt[:],
                        )
                        nc.scalar.dma_start(out_sh[ts(ob, P), ds(h * OC, OC)], osb[:])

    nc.compile()
    return nc


_CACHE = {}


def _get_nc(key):
    if key not in _CACHE:
        _CACHE[key] = build_nc(*key)
    return _CACHE[key]


def make_in_maps(x2d, weight, n_cores=N_CORES, tg=TG, fg=FG):
    """Host-side sharding: per-core pre-transposed inputs, x in fp8-e4m3."""
    import ml_dtypes

    t_tot, i_full = x2d.shape
    o_full = weight.shape[0]
    t_sh = t_tot // tg
    o_sh = o_full // fg
    i_slab = i_full // tg
    x_f8 = x2d.astype(ml_dtypes.float8_e4m3)
    wT_halves = {}
    for b in range(fg):
        wT_halves[b] = np.ascontiguousarray(weight[b * o_sh : (b + 1) * o_sh].T)
    in_maps = []
    for cid in range(n_cores):
        g, b = cid // fg, cid % fg
        # rotate i-rows so rows [0, i_slab) are this core's distinct |W|
        # slab; the contraction sum is invariant to the rotation as long
        # as xT rows are rotated identically.
        roll = -g * i_slab
        in_maps.append(
            {
                "xT": np.ascontiguousarray(
                    np.roll(x_f8[g * t_sh : (g + 1) * t_sh].T, roll, axis=0)
                ),
                "wT": np.roll(wT_halves[b], roll, axis=0),
            }
        )
    return in_maps


def run(x2d, weight, n_cores=N_CORES, tg=TG, fg=FG):
    """Run the sharded device computation: returns x @ q^T * s_w, [Ttot, O_full]."""
    from concourse.bass_utils import run_bass_kernel_spmd

    t_tot, i_full = x2d.shape
    o_full = weight.shape[0]
    t_sh = t_tot // tg
    o_sh = o_full // fg
    key = (t_sh, o_sh, i_full, n_cores, tg, o_full * i_full)
    nc = _get_nc(key)

    in_maps = make_in_maps(x2d, weight, n_cores, tg, fg)
    res = run_bass_kernel_spmd(nc, in_maps, core_ids=list(range(n_cores)))
    out = np.empty((t_tot, o_full), np.float32)
    for cid in range(n_cores):
        g, b = cid // fg, cid % fg
        out[g * t_sh : (g + 1) * t_sh, b * o_sh : (b + 1) * o_sh] = res.results[
            cid
        ]["out_sh"].T
    return out


def kernel(x, weight, bias):
    x = np.asarray(x, np.float32)
    weight = np.asarray(weight, np.float32)
    bias = np.asarray(bias, np.float32)
    t_tot = x.shape[0] * x.shape[1]
    out = run(x.reshape(t_tot, x.shape[2]), weight)
    # bias term: out += bias * s_x (exact reference semantics; zero for
    # this problem's bias). The matmul term is s_x-invariant.
    if np.any(bias):
        s_x = np.float32(max(np.mean(np.abs(x)), EPS))
        out = out + (bias * s_x)[None, :]
    return out.reshape(x.shape[0], x.shape[1], weight.shape[0])


# revision 15
# speedup vs baseline: 1.2066x; 1.0375x over previous
"""BitNetLinear (ternary eval-mode) forward on 8 trn2 NeuronCores.

Math (reference):
    s_w  = max(mean|W|, eps);  q = sign(W) * (|W/s_w| > 0.5)
    s_x  = max(mean|x|, eps)
    out  = (x/s_x) @ (q*s_w)^T * s_x + bias * s_x
         = x @ q^T * s_w + bias * s_x          (exact in real arithmetic)

Sharding: 2D grid, TG=2 token groups x FG=4 out-feature groups.
Each core: T=2048 tokens, O=1024 out features, I=4096 contraction
(FG=4 halves per-core W traffic and quantize work vs FG=2; PE work is
identical and stays the bottleneck).
Host passes x and W shards pre-transposed (i-major) with x cast to
fp8-e4m3 (measured end-to-end max rel err 0.017 < 2e-2 gate); W stays
f32 so the |w|>thr comparisons are exact.

Device pipeline per core (fp8 DoubleRow matmul):
  - phase S: |W| partial sum over this core's i-slab, read as 8 x 1MiB
    DMAs split across both hwdge rings with nothing else in flight (the
    slab doubles as the kb<4 strips of every o-chunk, so the 8 MB read
    is reused by quantization). Rolling DVE abs-reduce, gpsimd
    partition_all_reduce, then a 1-scalar ncfw AllReduce — the only
    collective; its ~50-90 us init/barrier/latency dominates the head,
    so all x / chunk-0 W prefetch is gated behind the reduction and
    runs on the ACT ring underneath the collective. The thr broadcast
    is issued after every prefetch so it cannot head-of-line-block a
    dma queue while it waits on the collective.
  - thr = 0.5*max(sum/N, eps) on chip.
  - quantize per [128,2,512] f32 strip into q in {-1,0,1} fp8 (2 ops;
    fp8 stores are only fast from f32 sources, and only on DVE/ACT):
        l  = Sigmoid(-1e30*(w + thr))     (ACT; exact {0,1} step)
        q8 = (w is_gt thr) - l            (DVE scalar_tensor_tensor)
  - matmul: fp8e4 DoubleRow, stationary q8 [128,2,128], moving x
    [128,2,512] -> psum [128 o, 512 t], 256-deep contraction per MM
    (measured 218.8 ns/MM = 2x bf16 FLOP rate, LDWEIGHTS fully hidden).
  - evict with scale s_w = 2*thr on ACT; output is o-major [O, T];
    host transposes back (layout only).
"""

import sys

sys.path.insert(0, "/opt/trn_rl_repo")

import numpy as np

P = 128
EPS = 1e-8
BIG = 1.0e30  # sigmoid step sharpener

B, S = 2, 2048
I_FULL = 4096  # in_features
O_FULL = 4096  # out_features
N_CORES = 8
TG, FG = 2, 4
T_SH = (B * S) // TG  # 2048
O_SH = O_FULL // FG  # 1024
KB = 256  # contraction rows per DoubleRow k-block
OC = 512  # o-chunk width (quantize granularity)


def build_nc(T, O, I, n_cores, tg, w_elems_total):
    """Build + compile the SPMD Bass module for one core shape."""
    from concourse import bacc, mybir, tile, bass_isa
    import concourse.bass as bass
    from concourse.bass import ts, ds

    f32 = mybir.dt.float32
    fp8 = mybir.dt.float8e4
    A = mybir.AluOpType
    DR = mybir.MatmulPerfMode.DoubleRow

    assert T % OC == 0 and O % OC == 0 and I % KB == 0
    n_kb = I // KB  # 16 k-blocks
    n_oc = O // OC  # 4 o-chunks
    n_th = T // OC  # 2 token halves
    slab_kb = (I // tg) // KB  # 4 k-blocks in this core's |W| slab

    nc = bacc.Bacc(
        "TRN2", target_bir_lowering=False, debug=False, num_devices=n_cores
    )
    xT = nc.dram_tensor("xT", [I, T], fp8, kind="ExternalInput").ap()
    wT = nc.dram_tensor("wT", [I, O], f32, kind="ExternalInput").ap()
    out_sh = nc.dram_tensor("out_sh", [O, T], f32, kind="ExternalOutput").ap()

    def src3(t_ap, off, d0, d1, d2):
        # 3D DRAM access pattern [(stride, count), ...], element units
        return bass.AP(
            tensor=t_ap.tensor,
            offset=t_ap.offset + off,
            ap=[list(d0), list(d1), list(d2)],
        )

    def w_src(kb, c):
        return src3(wT, kb * KB * O + c * OC, (O, P), (P * O, 2), (1, OC))

    with tile.TileContext(nc) as tc:
        with (
            tc.tile_pool(name="scal", bufs=1) as scal,
            tc.tile_pool(name="dram", bufs=1, space="DRAM") as dram,
            tc.tile_pool(name="slab", bufs=1) as slab_pool,
            tc.tile_pool(name="xp", bufs=1) as x_pool,
            tc.tile_pool(name="wsp", bufs=1) as ws_pool,
            tc.tile_pool(name="lp", bufs=1) as l_pool,
            tc.tile_pool(name="qp", bufs=1) as q_pool,
            tc.tile_pool(name="op", bufs=2) as o_pool,
            tc.tile_pool(name="ps", bufs=1, space="PSUM") as ps_pool,
        ):
            # ---- phase S: |W| partial over the slab (k-blocks 0..slab_kb
            # of every o-chunk; the host i-rolls wT per core so the slab
            # rows are distinct across the token group). Strips stay
            # resident — they are exactly the kb<slab_kb strips the
            # quantizer needs later.
            slab = {}
            acc = scal.tile([P, slab_kb], f32)
            for kb in range(slab_kb):
                wst = slab_pool.tile(
                    [P, 2, O], f32, tag=f"sl_{kb}", name=f"sl_{kb}"
                )
                # >=1MiB DMAs (each splits across its ring's 16 SDMA
                # slots), alternating the two hwdge rings; nothing else
                # is allowed on either ring until the slab lands
                eng = nc.sync if kb % 2 == 0 else nc.scalar
                eng.dma_start(
                    wst[:],
                    src3(wT, kb * KB * O, (O, P), (P * O, 2), (1, O)),
                )
                nc.vector.tensor_reduce(
                    acc[:, kb : kb + 1],
                    wst[:],
                    axis=mybir.AxisListType.XY,
                    op=A.add,
                    apply_absolute_value=True,
                )
                slab[kb] = wst
            red = scal.tile([P, 1], f32)
            nc.vector.tensor_reduce(red[:], acc[:], axis=mybir.AxisListType.X, op=A.add)
            ssum = scal.tile([P, 1], f32)
            nc.gpsimd.partition_all_reduce(
                ssum[:], red[:], channels=P, reduce_op=bass_isa.ReduceOp.add
            )

            # ---- the one collective: AllReduce of the scalar partial
            # (triggered as early as possible; the ncfw barrier's per-core
            # end time is runtime jitter we cannot control)
            cc_in = dram.tile([1, 1], f32)
            cc_out = dram.tile([1, 1], f32)
            nc.sync.dma_start(cc_in[:], ssum[0:1, 0:1])
            nc.gpsimd.collective_compute(
                "AllReduce",
                A.add,
                replica_groups=[list(range(n_cores))],
                ins=[cc_in[:]],
                outs=[cc_out[:]],
            )
            co = cc_out[:]

            # ---- x prefetch on the ACT ring, gated behind the slab
            # reduction: transfers within a ring overlap (FIFO is
            # start-order only), so an explicit dependency is the only
            # way to keep the slab read at full dual-ring bandwidth.
            gate = scal.tile([P, 1], f32)
            nc.scalar.activation(gate[:], red[:], mybir.ActivationFunctionType.Copy)
            x8 = []
            for kb in range(n_kb):
                xb = x_pool.tile([P, 2, T], fp8, tag=f"x_{kb}", name=f"x_{kb}")
                nc.scalar.dma_start(
                    xb[:], src3(xT, kb * KB * T, (T, P), (P * T, 2), (1, T))
                )
                x8.append(xb)

            # ---- chunk-0 streamed-strip prefetch (runs under the
            # collective head; WAR rotation handles chunks 1-3)
            def stream_strip(ci, kb, engine=None):
                wst = ws_pool.tile(
                    [P, 2, OC], f32, tag=f"ws_{kb}", name=f"ws_{ci}_{kb}"
                )
                (engine or nc.sync).dma_start(wst[:], w_src(kb, ci))
                return wst

            strips0 = {
                kb: stream_strip(0, kb, nc.scalar) for kb in range(slab_kb, n_kb)
            }

            # ---- thr broadcast, AFTER every prefetch issue in program
            # order (it waits on the collective; anything behind it on a
            # dma queue would head-of-line block until the AR completes)
            s_bc = scal.tile([P, 1], f32)
            nc.sync.dma_start(
                s_bc[:],
                bass.AP(tensor=co.tensor, offset=co.offset, ap=[[0, P], [1, 1]]),
            )
            # thr = 0.5*max(sum/N, EPS); bstep = -BIG*thr; swt = s_w = 2*thr.
            # All three directly from s_bc in one op each (parallel, off the
            # critical chain). N = 2^24 so the x2/x0.5 foldings are exact.
            thr = scal.tile([P, 1], f32)
            nc.vector.tensor_scalar(
                out=thr[:],
                in0=s_bc[:],
                scalar1=0.5 / float(w_elems_total),
                scalar2=0.5 * EPS,
                op0=A.mult,
                op1=A.max,
            )
            bstep = scal.tile([P, 1], f32)
            nc.vector.tensor_scalar(
                out=bstep[:],
                in0=s_bc[:],
                scalar1=-BIG * 0.5 / float(w_elems_total),
                scalar2=-BIG * 0.5 * EPS,
                op0=A.mult,
                op1=A.min,
            )
            swt = scal.tile([P, 1], f32)
            nc.vector.tensor_scalar(
                out=swt[:],
                in0=s_bc[:],
                scalar1=1.0 / float(w_elems_total),
                scalar2=EPS,
                op0=A.mult,
                op1=A.max,
            )

            # ---- main loop
            for ci in range(n_oc):
                qts = []
                for kb in range(n_kb):
                    if kb < slab_kb:
                        wst = slab[kb][:, :, ds(ci * OC, OC)]
                    elif ci == 0:
                        wst = strips0[kb][:]
                    else:
                        wst = stream_strip(ci, kb)[:]
                    # l = 1 iff w < -thr (sigmoid of -BIG*(w+thr): exact step)
                    lt = l_pool.tile(
                        [P, 2, OC], f32, tag=f"l_{kb % 2}", name=f"l_{ci}_{kb}"
                    )
                    nc.scalar.activation(
                        lt[:],
                        wst,
                        mybir.ActivationFunctionType.Sigmoid,
                        scale=-BIG,
                        bias=bstep[:],
                    )
                    # q8 = (w > thr) - l  in {-1, 0, 1}, straight to fp8
                    q8 = q_pool.tile(
                        [P, 2, OC], fp8, tag=f"q_{kb}_{ci % 2}", name=f"q_{ci}_{kb}"
                    )
                    nc.vector.scalar_tensor_tensor(
                        out=q8[:],
                        in0=wst,
                        scalar=thr[:],
                        op0=A.is_gt,
                        in1=lt[:],
                        op1=A.subtract,
                    )
                    qts.append(q8)
                for j in range(OC // P):
                    ob = ci * (OC // P) + j
                    pst = [
                        ps_pool.tile(
                            [P, OC],
                            f32,
                            tag=f"ps{(ob * n_th + h) % 8}",
                            name=f"ps_{ob}_{h}",
                        )
                        for h in range(n_th)
                    ]
                    for kb in range(n_kb):
                        lhs = qts[kb][:, :, ts(j, P)]
                        for h in range(n_th):
                            nc.tensor.matmul(
                                pst[h][:],
                                lhsT=lhs,
                                rhs=x8[kb][:, :, ds(h * OC, OC)],
                                start=(kb == 0),
                                stop=(kb == n_kb - 1),
                                perf_mode=DR,
                            )
                    for h in range(n_th):
                        osb = o_pool.tile([P, OC], f32, tag="o", name=f"o_{ob}_{h}")
                        # psum holds x @ q^T; scale by s_w
                        nc.scalar.activation(
                            osb[:],
                            pst[h][:],
                            mybir.ActivationFunctionType.Copy,
                            scale=swt[:],
                        )
                        nc.scalar.dma_start(out_sh[ts(ob, P), ds(h * OC, OC)], osb[:])

    nc.compile()
    return nc


_CACHE = {}


def _get_nc(key):
    if key not in _CACHE:
        _CACHE[key] = build_nc(*key)
    return _CACHE[key]


def make_in_maps(x2d, weight, n_cores=N_CORES, tg=TG, fg=FG):
    """Host-side sharding: per-core pre-transposed inputs, x in fp8-e4m3."""
    import ml_dtypes

    t_tot, i_full = x2d.shape
    o_full = weight.shape[0]
    t_sh = t_tot // tg
    o_sh = o_full // fg
    i_slab = i_full // tg
    x_f8 = x2d.astype(ml_dtypes.float8_e4m3)
    wT_halves = {}
    for b in range(fg):
        wT_halves[b] = np.ascontiguousarray(weight[b * o_sh : (b + 1) * o_sh].T)
    in_maps = []
    for cid in range(n_cores):
        g, b = cid // fg, cid % fg
        # rotate i-rows so rows [0, i_slab) are this core's distinct |W|
        # slab; the contraction sum is invariant to the rotation as long
        # as xT rows are rotated identically.
        roll = -g * i_slab
        in_maps.append(
            {
                "xT": np.ascontiguousarray(
                    np.roll(x_f8[g * t_sh : (g + 1) * t_sh].T, roll, axis=0)
                ),
                "wT": np.roll(wT_halves[b], roll, axis=0),
            }
        )
    return in_maps


def run(x2d, weight, n_cores=N_CORES, tg=TG, fg=FG):
    """Run the sharded device computation: returns x @ q^T * s_w, [Ttot, O_full]."""
    from concourse.bass_utils import run_bass_kernel_spmd

    t_tot, i_full = x2d.shape
    o_full = weight.shape[0]
    t_sh = t_tot // tg
    o_sh = o_full // fg
    key = (t_sh, o_sh, i_full, n_cores, tg, o_full * i_full)
    nc = _get_nc(key)

    in_maps = make_in_maps(x2d, weight, n_cores, tg, fg)
    res = run_bass_kernel_spmd(nc, in_maps, core_ids=list(range(n_cores)))
    out = np.empty((t_tot, o_full), np.float32)
    for cid in range(n_cores):
        g, b = cid // fg, cid % fg
        out[g * t_sh : (g + 1) * t_sh, b * o_sh : (b + 1) * o_sh] = res.results[
            cid
        ]["out_sh"].T
    return out


def kernel(x, weight, bias):
    x = np.asarray(x, np.float32)
    weight = np.asarray(weight, np.float32)
    bias = np.asarray(bias, np.float32)
    t_tot = x.shape[0] * x.shape[1]
    out = run(x.reshape(t_tot, x.shape[2]), weight)
    # bias term: out += bias * s_x (exact reference semantics; zero for
    # this problem's bias). The matmul term is s_x-invariant.
    if np.any(bias):
        s_x = np.float32(max(np.mean(np.abs(x)), EPS))
        out = out + (bias * s_x)[None, :]
    return out.reshape(x.shape[0], x.shape[1], weight.shape[0])
